# revision 12
# baseline (speedup 1.0000x reference)
"""ALiBi causal multi-head attention on 8 Trainium2 NeuronCores.

Module: qkv = x @ qkv_w + qkv_b; per-head causal attention with ALiBi bias
(slope_i = 2^(-i/2), 16 heads, head_size 128, no 1/sqrt(d) scale);
out = ctx @ out_w + out_b.  x: [2, 2048, 2048] fp32.

Sharding (8 cores): core c -> (batch b = c//4, head group g = c%4 of 4 heads).
Each core: qkv projection for its 4 heads, attention, and a partial output
projection (its heads' rows of out_w).  Host sums the 4 partials per batch
and adds out_b — no device collectives.

Device algorithm per core (all matmuls fp32r: 4x fp32 rate, ~1.5e-4 rel err):
  phase 1 (single pass over x^T): qT,kT = w^T @ x^T (transposed projection,
           feature-major), v = x @ wv (seq-major).  qT spilled to DRAM
           (SBUF pressure), kT+v resident in SBUF.
  phase 2: per (head, 512-query superblock): scoresT[k,q] tiles via PE;
           ALiBi+causal bias tiles = slope * host pattern; DVE adds bias,
           ACT exps; denominators via all-ones [128,128] PE matmuls
           accumulated in PSUM (every partition holds the key-sum);
           AV matmuls accumulate ctxT[d,q]; DVE reciprocal+mult normalizes.
  phase 3: out_partial[s, hid] = ctxT^T @ wo_rows, streamed to DRAM.

softmax runs without max-subtraction: scores = q.k + alibi <= ~72 here
(0.02-scaled weights), far inside fp32 exp range.
"""

import os
from contextlib import ExitStack

import numpy as np

import concourse.bass as bass
import concourse.mybir as mybir
import concourse.tile as tile
from concourse import bacc
from concourse.bass_utils import run_bass_kernel_spmd

F32 = mybir.dt.float32
F32R = mybir.dt.float32r
AF = mybir.ActivationFunctionType
OP = mybir.AluOpType

NUM_HEADS = 16
HEAD = 128
S = 2048
HID = 2048
B = 2
N_CORES = 8
HPG = 4  # heads per group (per core)
GF = HPG * HEAD  # 512 features per group
SN = 512  # phase-1 seq tile
QSB = 512  # query superblock
MASK_NEG = -1.0e5

_SLOPES = np.asarray([1.0 / 2.0 ** (i / 2.0) for i in range(NUM_HEADS)], np.float32)

_NC_CACHE = {}


def _build_nc():
    nc = bacc.Bacc("TRN2", target_bir_lowering=False)

    xt = nc.declare_dram_parameter("xt", [HID, S], F32, isOutput=False)
    wqk = nc.declare_dram_parameter("wqk", [HID, 2 * GF], F32, isOutput=False)
    wv = nc.declare_dram_parameter("wv", [HID, GF], F32, isOutput=False)
    wo = nc.declare_dram_parameter("wo", [GF, HID], F32, isOutput=False)
    bqk = nc.declare_dram_parameter("bqk", [128, 8], F32, isOutput=False)
    bvb = nc.declare_dram_parameter("bvb", [128, GF], F32, isOutput=False)
    patd = nc.declare_dram_parameter("patd", [128, QSB], F32, isOutput=False)
    patm = nc.declare_dram_parameter("patm", [4, 128, QSB], F32, isOutput=False)
    patms = nc.declare_dram_parameter("patms", [4, 128, QSB], F32, isOutput=False)
    acol = nc.declare_dram_parameter("acol", [128, HPG, 13], F32, isOutput=False)
    slopes = nc.declare_dram_parameter("slopes", [128, HPG], F32, isOutput=False)
    mconst = nc.declare_dram_parameter("mconst", [128, HPG, 16], F32, isOutput=False)
    ones_m = nc.declare_dram_parameter("ones_m", [128, 128], F32, isOutput=False)
    yp = nc.declare_dram_parameter("yp", [S, HID], F32, isOutput=True)

    qspill = nc.dram_tensor("qspill", [HPG, 128, S], F32)

    xt_t = xt.rearrange("(ko p) s -> p ko s", p=128)
    wqk_t = wqk.rearrange("(ko p) m -> p ko m", p=128)
    wv_t = wv.rearrange("(ko p) f -> p ko f", p=128)
    wo_t = wo.rearrange("(ho p) n -> p ho n", p=128)

    KO = HID // 128  # 16
    NT1 = S // SN  # 4
    KC = 4  # ko chunks
    KPC = KO // KC  # 4 ko per chunk

    with tile.TileContext(nc) as tc, ExitStack() as ctx:
        persist = ctx.enter_context(tc.tile_pool(name="persist", bufs=1))
        k_sb = persist.tile([128, HPG, S], F32R, tag="k_sb")
        v_sb = persist.tile([128, S // 128, GF], F32R, tag="v_sb")

        bqk_sb = persist.tile([128, 8], F32, tag="bqk")
        nc.sync.dma_start(bqk_sb[:], bqk[:, :])
        bvb_sb = persist.tile([128, GF], F32, tag="bvb")
        nc.sync.dma_start(bvb_sb[:], bvb[:, :])
        ones_sb = persist.tile([128, 128], F32R, tag="ones_m")
        nc.sync.dma_start(ones_sb[:], ones_m[:, :].bitcast(F32R))
        patms_sb = persist.tile([128, 4, QSB], F32, tag="patms")
        nc.sync.dma_start(patms_sb[:], patms.rearrange("r p s -> p r s"))
        slopes_sb = persist.tile([128, HPG], F32, tag="slopes")
        nc.sync.dma_start(slopes_sb[:], slopes[:, :])
        acol_sb = persist.tile([128, HPG, 13], F32, tag="acol")
        nc.sync.dma_start(acol_sb[:], acol[:, :, :])

        # ---------------- phase 1: projections (single pass over x) --------
        with (
            tc.tile_pool(name="p1w", bufs=1) as p1w,
            tc.tile_pool(name="p1wv", bufs=4) as p1wv,
            tc.tile_pool(name="p1x2", bufs=2) as p1x2,
            tc.tile_pool(name="p1x1", bufs=1) as p1x1,
            tc.tile_pool(name="p1o", bufs=2) as p1o,
            tc.tile_pool(name="ps1", bufs=3, space="PSUM") as ps1,
            tc.tile_pool(name="ps1v", bufs=1, space="PSUM") as ps1v,
        ):
            whc = []
            for c in range(KC):
                wt = p1w.tile([128, KPC, 2 * GF], F32R, tag=f"wqk{c}")
                nc.sync.dma_start(
                    wt[:], wqk_t[:, c * KPC : (c + 1) * KPC, :].bitcast(F32R)
                )
                whc.append(wt)
            for n in range(NT1):
                xnc = []
                for c in range(KC):
                    xpool = p1x2 if c < 3 else p1x1
                    xct = xpool.tile(
                        [128, KPC, SN], F32R, tag=f"xn{c}", name=f"xn{c}"
                    )
                    nc.sync.dma_start(
                        xct[:],
                        xt_t[
                            :, c * KPC : (c + 1) * KPC, n * SN : (n + 1) * SN
                        ].bitcast(F32R),
                    )
                    xnc.append(xct)
                # v first: needs only one wv ko-slice + one x chunk to start
                psvs = [
                    ps1v.tile([128, GF], F32, tag=f"psv{ms}", name=f"psv{ms}")
                    for ms in range(SN // 128)
                ]
                for ko in range(KO):
                    wvk = p1wv.tile([128, GF], F32R, tag="wvk")
                    nc.sync.dma_start(wvk[:], wv_t[:, ko, :].bitcast(F32R))
                    c, kk = divmod(ko, KPC)
                    for ms in range(SN // 128):
                        nc.tensor.matmul(
                            psvs[ms][:],
                            xnc[c][:, kk, ms * 128 : (ms + 1) * 128],
                            wvk[:],
                            start=(ko == 0),
                            stop=(ko == KO - 1),
                        )
                for ms in range(SN // 128):
                    nc.vector.tensor_tensor(
                        v_sb[:, n * (SN // 128) + ms, :],
                        psvs[ms][:],
                        bvb_sb[:],
                        OP.add,
                    )
                for m in range(2 * HPG):  # 0-3: q heads, 4-7: k heads
                    psq = ps1.tile([128, SN], F32, tag="psq")
                    for ko in range(KO):
                        c, kk = divmod(ko, KPC)
                        nc.tensor.matmul(
                            psq[:],
                            whc[c][:, kk, m * 128 : (m + 1) * 128],
                            xnc[c][:, kk, :],
                            start=(ko == 0),
                            stop=(ko == KO - 1),
                        )
                    bcol = bqk_sb[:, m : m + 1]
                    if m < HPG:
                        qo = p1o.tile([128, SN], F32, tag="qo")
                        nc.scalar.activation(qo[:], psq[:], AF.Identity, bias=bcol)
                        nc.sync.dma_start(qspill[m, :, n * SN : (n + 1) * SN], qo[:])
                    else:
                        nc.scalar.activation(
                            k_sb[:, m - HPG, n * SN : (n + 1) * SN],
                            psq[:],
                            AF.Identity,
                            bias=bcol,
                        )

        # ------- phases 2+3 share ctx / wo pools (wo prefetches early) -----
        NQSB = S // QSB  # 4
        with (
            tc.tile_pool(name="ctxp", bufs=1) as ctxp,
            tc.tile_pool(name="p3w", bufs=1) as p3w,
        ):
            ctx_sb = ctxp.tile([128, HPG, S], F32R, tag="ctx_sb")
            wo_sb = p3w.tile([128, HPG, HID], F32R, tag="wo")
            nc.sync.dma_start(wo_sb[:], wo_t.bitcast(F32R))

            # ---------------- phase 2: attention ----------------
            with (
                tc.tile_pool(name="consts2", bufs=1) as c2p,
                tc.tile_pool(name="qt", bufs=2) as qtp,
                tc.tile_pool(name="biasp", bufs=1) as biasp,
                tc.tile_pool(name="stp", bufs=2) as stp,
                tc.tile_pool(name="atp", bufs=5) as atp,
                tc.tile_pool(name="ssp", bufs=2) as ssp,
                tc.tile_pool(name="pss", bufs=4, space="PSUM") as pss,
                tc.tile_pool(name="pssum", bufs=2, space="PSUM") as pssum,
                tc.tile_pool(name="psctx", bufs=2, space="PSUM") as psctx,
            ):
                patd_sb = c2p.tile([128, QSB], F32, tag="patd")
                nc.sync.dma_start(patd_sb[:], patd[:, :])
                patm_sb = c2p.tile([128, 4, QSB], F32, tag="patm")
                nc.sync.dma_start(patm_sb[:], patm.rearrange("r p s -> p r s"))
                mconst_sb = c2p.tile([128, HPG, 16], F32, tag="mconst")
                nc.sync.dma_start(mconst_sb[:], mconst[:, :, :])

                for h in (2, 3, 0, 1):
                    shift_path = h >= 2  # small-slope slots: per-key ACT bias
                    bias_tiles = {}
                    psrc = patms_sb if shift_path else patm_sb
                    for r in range(4):
                        bt = biasp.tile([128, QSB], F32, tag=f"biasr{r}")
                        nc.vector.tensor_scalar_mul(
                            bt[:], psrc[:, r, :], slopes_sb[:, h : h + 1]
                        )
                        bias_tiles[-r] = bt
                    if not shift_path:
                        for m in range(1, 13):
                            bt = biasp.tile([128, QSB], F32, tag=f"biasm{m}")
                            nc.vector.tensor_scalar(
                                bt[:],
                                patd_sb[:],
                                slopes_sb[:, h : h + 1],
                                mconst_sb[:, h, m : m + 1],
                                OP.mult,
                                OP.add,
                            )
                            bias_tiles[m] = bt
                    for qsb in range(NQSB):
                        qt = qtp.tile([128, QSB], F32R, tag="qt")
                        nc.sync.dma_start(
                            qt[:],
                            qspill[h, :, qsb * QSB : (qsb + 1) * QSB].bitcast(F32R),
                        )
                        kmax = 4 * qsb + 3
                        # slot 0 holds the largest slopes: key blocks with
                        # m >= 5 contribute exp(<= -60) relative -> skip
                        kj_min = max(0, 4 * qsb - 4) if h == 0 else 0
                        ps_sum = pssum.tile([128, QSB], F32, tag="pssum")
                        ps_ctx = psctx.tile([128, QSB], F32, tag="psctx")
                        for kj in range(kj_min, kmax + 1):
                            m = 4 * qsb - kj
                            ps_s = pss.tile([128, QSB], F32, tag="pss")
                            nc.tensor.matmul(
                                ps_s[:],
                                k_sb[:, h, kj * 128 : (kj + 1) * 128],
                                qt[:],
                                start=True,
                                stop=True,
                            )
                            at = atp.tile([128, QSB], F32R, tag="at")
                            if shift_path and m >= 1:
                                nc.scalar.activation(
                                    at[:], ps_s[:], AF.Exp,
                                    bias=acol_sb[:, h, m : m + 1],
                                )
                            else:
                                st = stp.tile([128, QSB], F32, tag="st")
                                nc.vector.tensor_tensor(
                                    st[:], ps_s[:], bias_tiles[m][:], OP.add
                                )
                                nc.scalar.activation(at[:], st[:], AF.Exp)
                            nc.tensor.matmul(
                                ps_sum[:],
                                ones_sb[:],
                                at[:],
                                start=(kj == kj_min),
                                stop=(kj == kmax),
                            )
                            nc.tensor.matmul(
                                ps_ctx[:],
                                v_sb[:, kj, h * 128 : (h + 1) * 128],
                                at[:],
                                start=(kj == kj_min),
                                stop=(kj == kmax),
                            )
                        ss = ssp.tile([128, QSB], F32, tag="ss")
                        nc.vector.tensor_copy(ss[:], ps_sum[:])
                        rb = ssp.tile([128, QSB], F32, tag="rb")
                        nc.vector.reciprocal(rb[:], ss[:])
                        nc.vector.tensor_tensor(
                            ctx_sb[:, h, qsb * QSB : (qsb + 1) * QSB],
                            ps_ctx[:],
                            rb[:],
                            OP.mult,
                        )

            # ---------------- phase 3: output projection ----------------
            with (
                tc.tile_pool(name="p3o", bufs=4) as p3o,
                tc.tile_pool(name="ps3", bufs=2, space="PSUM") as ps3,
            ):
                for ms in range(S // 128):
                    psos = [
                        ps3.tile([128, 512], F32, tag=f"pso{nt}", name=f"pso{nt}")
                        for nt in range(HID // 512)
                    ]
                    for h in range(HPG):
                        for nt in range(HID // 512):
                            nc.tensor.matmul(
                                psos[nt][:],
                                ctx_sb[:, h, ms * 128 : (ms + 1) * 128],
                                wo_sb[:, h, nt * 512 : (nt + 1) * 512],
                                start=(h == 0),
                                stop=(h == HPG - 1),
                            )
                    for nt in range(HID // 512):
                        osb = p3o.tile([128, 512], F32, tag="osb")
                        nc.vector.tensor_copy(osb[:], psos[nt][:])
                        nc.sync.dma_start(
                            yp[
                                ms * 128 : (ms + 1) * 128,
                                nt * 512 : (nt + 1) * 512,
                            ],
                            osb[:],
                        )

    nc.compile()
    return nc


def _host_inputs(x, qkv_w, qkv_b, out_w):
    """Per-core input dicts."""
    jj = np.arange(128, dtype=np.float32)[:, None]
    ii = np.arange(QSB, dtype=np.float32)[None, :]
    d_pat = np.ascontiguousarray(jj - ii)  # [128, 512]
    patm = np.empty((4, 128, QSB), np.float32)
    patms = np.empty((4, 128, QSB), np.float32)
    for r in range(4):
        v = d_pat + 128.0 * r
        patm[r] = np.where(v <= 0.0, v, MASK_NEG)
        vs = jj + 128.0 * r - 511.0  # slope*(j - i_max), qsb-independent
        patms[r] = np.where(v <= 0.0, vs, MASK_NEG)
    ones_m = np.ones((128, 128), np.float32)

    in_maps = []
    for c in range(N_CORES):
        b, g = divmod(c, HPG)
        xt = np.ascontiguousarray(x[b].T)
        heads = [4 * j + g for j in range(HPG)]  # slot j -> head 4j+g
        hcols = np.concatenate(
            [np.arange(hh * 128, (hh + 1) * 128) for hh in heads]
        )
        qcols = hcols
        kcols = HID + hcols
        vcols = 2 * HID + hcols
        wqk = np.ascontiguousarray(
            np.concatenate([qkv_w[:, qcols], qkv_w[:, kcols]], axis=1)
        )
        wv = np.ascontiguousarray(qkv_w[:, vcols])
        wo = np.ascontiguousarray(out_w[hcols, :])
        bqk_v = np.concatenate([qkv_b[qcols], qkv_b[kcols]])
        bqk = np.ascontiguousarray(bqk_v.reshape(8, 128).T)
        bvb = np.ascontiguousarray(np.broadcast_to(qkv_b[vcols], (128, GF)))
        sl = _SLOPES[heads]
        slopes = np.ascontiguousarray(np.broadcast_to(sl, (128, HPG)))
        mc = np.empty((HPG, 16), np.float32)
        for h in range(HPG):
            for m in range(16):
                mc[h, m] = -128.0 * m * sl[h]
        mconst = np.ascontiguousarray(np.broadcast_to(mc, (128, HPG, 16)))
        jjv = np.arange(128, dtype=np.float32)
        ac = np.empty((128, HPG, 13), np.float32)
        for h in range(HPG):
            for m in range(13):
                ac[:, h, m] = sl[h] * (jjv - 511.0 - 128.0 * m)
        acol_a = np.ascontiguousarray(ac)
        in_maps.append(
            {
                "xt": xt,
                "wqk": wqk,
                "wv": wv,
                "wo": wo,
                "bqk": bqk,
                "bvb": bvb,
                "patd": d_pat,
                "patm": patm,
                "patms": patms,
                "acol": acol_a,
                "slopes": slopes,
                "mconst": mconst,
                "ones_m": ones_m,
            }
        )
    return in_maps


def kernel(x, qkv_w, qkv_b, out_w, out_b):
    x = np.asarray(x, np.float32)
    qkv_w = np.asarray(qkv_w, np.float32)
    qkv_b = np.asarray(qkv_b, np.float32)
    out_w = np.asarray(out_w, np.float32)
    out_b = np.asarray(out_b, np.float32)

    if "nc" not in _NC_CACHE:
        _NC_CACHE["nc"] = _build_nc()
    nc = _NC_CACHE["nc"]

    in_maps = _host_inputs(x, qkv_w, qkv_b, out_w)

    trace = bool(int(os.environ.get("BASS_ATTN_TRACE", "0")))
    if trace:
        import trace_shim  # noqa: F401

    res = run_bass_kernel_spmd(
        nc, in_maps, core_ids=list(range(N_CORES)), trace=trace
    )
    if trace and res.exec_time_ns is not None:
        print(f"HW exec time: {res.exec_time_ns} ns")
        _NC_CACHE["exec_time_ns"] = res.exec_time_ns
        _NC_CACHE["trace"] = (
            res.instructions_and_trace[1] if res.instructions_and_trace else None
        )

    out = np.empty((B, S, HID), np.float32)
    for b in range(B):
        acc = np.zeros((S, HID), np.float64)
        for g in range(HPG):
            acc += res.results[b * HPG + g]["yp"].astype(np.float64)
        out[b] = (acc + out_b.astype(np.float64)).astype(np.float32)
    return out


# revision 13
# speedup vs baseline: 1.1548x; 1.1548x over previous
"""ALiBi causal multi-head attention on 8 Trainium2 NeuronCores.

Module: qkv = x @ qkv_w + qkv_b; per-head causal attention with ALiBi bias
(slope_i = 2^(-i/2), 16 heads, head_size 128, no 1/sqrt(d) scale);
out = ctx @ out_w + out_b.  x: [2, 2048, 2048] fp32.

Sharding (8 cores): core c -> (batch b = c//4, head group g = c%4 of 4 heads).
Each core: qkv projection for its 4 heads, attention, and a partial output
projection (its heads' rows of out_w).  Host sums the 4 partials per batch
and adds out_b — no device collectives.

Device algorithm per core (all matmuls fp32r: 4x fp32 rate, ~1.5e-4 rel err):
  phase 1 (single pass over x^T): qT,kT = w^T @ x^T (transposed projection,
           feature-major), v = x @ wv (seq-major).  qT spilled to DRAM
           (SBUF pressure), kT+v resident in SBUF.
  phase 2: per (head, 512-query superblock): scoresT[k,q] tiles via PE;
           ALiBi+causal bias tiles = slope * host pattern; DVE adds bias,
           ACT exps; denominators via all-ones [128,128] PE matmuls
           accumulated in PSUM (every partition holds the key-sum);
           AV matmuls accumulate ctxT[d,q]; DVE reciprocal+mult normalizes.
  phase 3: out_partial[s, hid] = ctxT^T @ wo_rows, streamed to DRAM.

softmax runs without max-subtraction: scores = q.k + alibi <= ~72 here
(0.02-scaled weights), far inside fp32 exp range.
"""

import os
from contextlib import ExitStack

import numpy as np

import concourse.bass as bass
import concourse.mybir as mybir
import concourse.tile as tile
from concourse import bacc
from concourse.bass_utils import run_bass_kernel_spmd

F32 = mybir.dt.float32
F32R = mybir.dt.float32r
AF = mybir.ActivationFunctionType
OP = mybir.AluOpType

NUM_HEADS = 16
HEAD = 128
S = 2048
HID = 2048
B = 2
N_CORES = 8
HPG = 4  # heads per group (per core)
GF = HPG * HEAD  # 512 features per group
SN = 512  # phase-1 seq tile
QSB = 512  # query superblock
MASK_NEG = -1.0e5

_SLOPES = np.asarray([1.0 / 2.0 ** (i / 2.0) for i in range(NUM_HEADS)], np.float32)

_NC_CACHE = {}


def _build_nc():
    nc = bacc.Bacc("TRN2", target_bir_lowering=False)

    xt = nc.declare_dram_parameter("xt", [HID, S], F32, isOutput=False)
    wqk = nc.declare_dram_parameter("wqk", [HID, 2 * GF], F32, isOutput=False)
    wv = nc.declare_dram_parameter("wv", [HID, GF], F32, isOutput=False)
    wo = nc.declare_dram_parameter("wo", [GF, HID], F32, isOutput=False)
    bqk = nc.declare_dram_parameter("bqk", [128, 8], F32, isOutput=False)
    bvb = nc.declare_dram_parameter("bvb", [128, GF], F32, isOutput=False)
    patd = nc.declare_dram_parameter("patd", [128, QSB], F32, isOutput=False)
    patm = nc.declare_dram_parameter("patm", [4, 128, QSB], F32, isOutput=False)
    patms = nc.declare_dram_parameter("patms", [4, 128, QSB], F32, isOutput=False)
    acol = nc.declare_dram_parameter("acol", [128, HPG, 13], F32, isOutput=False)
    slopes = nc.declare_dram_parameter("slopes", [128, HPG], F32, isOutput=False)
    mconst = nc.declare_dram_parameter("mconst", [128, HPG, 16], F32, isOutput=False)
    ones_m = nc.declare_dram_parameter("ones_m", [128, 128], F32, isOutput=False)
    yp = nc.declare_dram_parameter("yp", [S, HID], F32, isOutput=True)

    qspill = nc.dram_tensor("qspill", [HPG, 128, S], F32)

    xt_t = xt.rearrange("(ko p) s -> p ko s", p=128)
    wqk_t = wqk.rearrange("(ko p) m -> p ko m", p=128)
    wv_t = wv.rearrange("(ko p) f -> p ko f", p=128)
    wo_t = wo.rearrange("(ho p) n -> p ho n", p=128)

    KO = HID // 128  # 16
    NT1 = S // SN  # 4
    KC = 4  # ko chunks
    KPC = KO // KC  # 4 ko per chunk

    with tile.TileContext(nc) as tc, ExitStack() as ctx:
        persist = ctx.enter_context(tc.tile_pool(name="persist", bufs=1))
        k_sb = persist.tile([128, HPG, S], F32R, tag="k_sb")
        v_sb = persist.tile([128, S // 128, GF], F32R, tag="v_sb")

        bqk_sb = persist.tile([128, 8], F32, tag="bqk")
        nc.sync.dma_start(bqk_sb[:], bqk[:, :])
        bvb_sb = persist.tile([128, GF], F32, tag="bvb")
        nc.sync.dma_start(bvb_sb[:], bvb[:, :])
        ones_sb = persist.tile([128, 128], F32R, tag="ones_m")
        nc.sync.dma_start(ones_sb[:], ones_m[:, :].bitcast(F32R))

        # ---------------- phase 1: projections (single pass over x) --------
        with (
            tc.tile_pool(name="p1w", bufs=1) as p1w,
            tc.tile_pool(name="p1wv", bufs=4) as p1wv,
            tc.tile_pool(name="p1x2", bufs=2) as p1x2,
            tc.tile_pool(name="p1x1", bufs=1) as p1x1,
            tc.tile_pool(name="p1o", bufs=2) as p1o,
            tc.tile_pool(name="ps1", bufs=3, space="PSUM") as ps1,
            tc.tile_pool(name="ps1v", bufs=1, space="PSUM") as ps1v,
        ):
            whc = []
            for c in range(KC):
                wt = p1w.tile([128, KPC, 2 * GF], F32R, tag=f"wqk{c}")
                nc.sync.dma_start(
                    wt[:], wqk_t[:, c * KPC : (c + 1) * KPC, :].bitcast(F32R)
                )
                whc.append(wt)
            for n in range(NT1):
                xnc = []
                for c in range(KC):
                    xpool = p1x2 if c < 3 else p1x1
                    xct = xpool.tile(
                        [128, KPC, SN], F32R, tag=f"xn{c}", name=f"xn{c}"
                    )
                    nc.sync.dma_start(
                        xct[:],
                        xt_t[
                            :, c * KPC : (c + 1) * KPC, n * SN : (n + 1) * SN
                        ].bitcast(F32R),
                    )
                    xnc.append(xct)
                # v first: needs only one wv ko-slice + one x chunk to start
                psvs = [
                    ps1v.tile([128, GF], F32, tag=f"psv{ms}", name=f"psv{ms}")
                    for ms in range(SN // 128)
                ]
                for ko in range(KO):
                    wvk = p1wv.tile([128, GF], F32R, tag="wvk")
                    nc.sync.dma_start(wvk[:], wv_t[:, ko, :].bitcast(F32R))
                    c, kk = divmod(ko, KPC)
                    for ms in range(SN // 128):
                        nc.tensor.matmul(
                            psvs[ms][:],
                            xnc[c][:, kk, ms * 128 : (ms + 1) * 128],
                            wvk[:],
                            start=(ko == 0),
                            stop=(ko == KO - 1),
                        )
                for ms in range(SN // 128):
                    nc.vector.tensor_tensor(
                        v_sb[:, n * (SN // 128) + ms, :],
                        psvs[ms][:],
                        bvb_sb[:],
                        OP.add,
                    )
                for m in range(2 * HPG):  # 0-3: q heads, 4-7: k heads
                    psq = ps1.tile([128, SN], F32, tag="psq")
                    for ko in range(KO):
                        c, kk = divmod(ko, KPC)
                        nc.tensor.matmul(
                            psq[:],
                            whc[c][:, kk, m * 128 : (m + 1) * 128],
                            xnc[c][:, kk, :],
                            start=(ko == 0),
                            stop=(ko == KO - 1),
                        )
                    bcol = bqk_sb[:, m : m + 1]
                    if m < HPG:
                        qo = p1o.tile([128, SN], F32, tag="qo")
                        nc.scalar.activation(qo[:], psq[:], AF.Identity, bias=bcol)
                        nc.sync.dma_start(qspill[m, :, n * SN : (n + 1) * SN], qo[:])
                    else:
                        nc.scalar.activation(
                            k_sb[:, m - HPG, n * SN : (n + 1) * SN],
                            psq[:],
                            AF.Identity,
                            bias=bcol,
                        )

        # ------- phases 2+3 share ctx / wo pools (wo prefetches early) -----
        NQSB = S // QSB  # 4
        with (
            tc.tile_pool(name="ctxp", bufs=1) as ctxp,
            tc.tile_pool(name="p3w", bufs=1) as p3w,
        ):
            ctx_sb = ctxp.tile([128, HPG, S], F32R, tag="ctx_sb")
            wo_sb = p3w.tile([128, HPG, HID], F32R, tag="wo")
            nc.sync.dma_start(wo_sb[:], wo_t.bitcast(F32R))

            # ---------------- phase 2: attention ----------------
            with (
                tc.tile_pool(name="consts2", bufs=1) as c2p,
                tc.tile_pool(name="qt", bufs=2) as qtp,
                tc.tile_pool(name="biasp", bufs=1) as biasp,
                tc.tile_pool(name="stp", bufs=2) as stp,
                tc.tile_pool(name="atp", bufs=5) as atp,
                tc.tile_pool(name="ssp", bufs=2) as ssp,
                tc.tile_pool(name="pss", bufs=4, space="PSUM") as pss,
                tc.tile_pool(name="pssum", bufs=2, space="PSUM") as pssum,
                tc.tile_pool(name="psctx", bufs=2, space="PSUM") as psctx,
            ):
                patd_sb = c2p.tile([128, QSB], F32, tag="patd")
                nc.sync.dma_start(patd_sb[:], patd[:, :])
                patm_sb = c2p.tile([128, 4, QSB], F32, tag="patm")
                nc.sync.dma_start(patm_sb[:], patm.rearrange("r p s -> p r s"))
                mconst_sb = c2p.tile([128, HPG, 16], F32, tag="mconst")
                nc.sync.dma_start(mconst_sb[:], mconst[:, :, :])
                slopes_sb = c2p.tile([128, HPG], F32, tag="slopes")
                nc.sync.dma_start(slopes_sb[:], slopes[:, :])
                patms_sb = c2p.tile([128, 4, QSB], F32, tag="patms")
                nc.sync.dma_start(patms_sb[:], patms.rearrange("r p s -> p r s"))
                acol_sb = c2p.tile([128, HPG, 13], F32, tag="acol")
                nc.sync.dma_start(acol_sb[:], acol[:, :, :])

                for h in range(HPG):
                    shift_path = h >= 2  # small-slope slots: per-key ACT bias
                    bias_tiles = {}
                    psrc = patms_sb if shift_path else patm_sb
                    for r in range(4):
                        bt = biasp.tile([128, QSB], F32, tag=f"biasr{r}")
                        nc.vector.tensor_scalar_mul(
                            bt[:], psrc[:, r, :], slopes_sb[:, h : h + 1]
                        )
                        bias_tiles[-r] = bt
                    if not shift_path:
                        for m in range(1, 13):
                            bt = biasp.tile([128, QSB], F32, tag=f"biasm{m}")
                            nc.vector.tensor_scalar(
                                bt[:],
                                patd_sb[:],
                                slopes_sb[:, h : h + 1],
                                mconst_sb[:, h, m : m + 1],
                                OP.mult,
                                OP.add,
                            )
                            bias_tiles[m] = bt
                    for qsb in range(NQSB):
                        qt = qtp.tile([128, QSB], F32R, tag="qt")
                        nc.sync.dma_start(
                            qt[:],
                            qspill[h, :, qsb * QSB : (qsb + 1) * QSB].bitcast(F32R),
                        )
                        kmax = 4 * qsb + 3
                        ps_sum = pssum.tile([128, QSB], F32, tag="pssum")
                        ps_ctx = psctx.tile([128, QSB], F32, tag="psctx")
                        for kj in range(kmax + 1):
                            m = 4 * qsb - kj
                            ps_s = pss.tile([128, QSB], F32, tag="pss")
                            nc.tensor.matmul(
                                ps_s[:],
                                k_sb[:, h, kj * 128 : (kj + 1) * 128],
                                qt[:],
                                start=True,
                                stop=True,
                            )
                            at = atp.tile([128, QSB], F32R, tag="at")
                            if shift_path and m >= 1:
                                nc.scalar.activation(
                                    at[:], ps_s[:], AF.Exp,
                                    bias=acol_sb[:, h, m : m + 1],
                                )
                            else:
                                st = stp.tile([128, QSB], F32, tag="st")
                                nc.vector.tensor_tensor(
                                    st[:], ps_s[:], bias_tiles[m][:], OP.add
                                )
                                nc.scalar.activation(at[:], st[:], AF.Exp)
                            nc.tensor.matmul(
                                ps_sum[:],
                                ones_sb[:],
                                at[:],
                                start=(kj == 0),
                                stop=(kj == kmax),
                            )
                            nc.tensor.matmul(
                                ps_ctx[:],
                                v_sb[:, kj, h * 128 : (h + 1) * 128],
                                at[:],
                                start=(kj == 0),
                                stop=(kj == kmax),
                            )
                        ss = ssp.tile([128, QSB], F32, tag="ss")
                        nc.vector.tensor_copy(ss[:], ps_sum[:])
                        rb = ssp.tile([128, QSB], F32, tag="rb")
                        nc.vector.reciprocal(rb[:], ss[:])
                        nc.vector.tensor_tensor(
                            ctx_sb[:, h, qsb * QSB : (qsb + 1) * QSB],
                            ps_ctx[:],
                            rb[:],
                            OP.mult,
                        )

            # ---------------- phase 3: output projection ----------------
            with (
                tc.tile_pool(name="p3o", bufs=4) as p3o,
                tc.tile_pool(name="ps3", bufs=2, space="PSUM") as ps3,
            ):
                for ms in range(S // 128):
                    psos = [
                        ps3.tile([128, 512], F32, tag=f"pso{nt}", name=f"pso{nt}")
                        for nt in range(HID // 512)
                    ]
                    for h in range(HPG):
                        for nt in range(HID // 512):
                            nc.tensor.matmul(
                                psos[nt][:],
                                ctx_sb[:, h, ms * 128 : (ms + 1) * 128],
                                wo_sb[:, h, nt * 512 : (nt + 1) * 512],
                                start=(h == 0),
                                stop=(h == HPG - 1),
                            )
                    for nt in range(HID // 512):
                        osb = p3o.tile([128, 512], F32, tag="osb")
                        nc.vector.tensor_copy(osb[:], psos[nt][:])
                        nc.sync.dma_start(
                            yp[
                                ms * 128 : (ms + 1) * 128,
                                nt * 512 : (nt + 1) * 512,
                            ],
                            osb[:],
                        )

    nc.compile()
    return nc


def _host_inputs(x, qkv_w, qkv_b, out_w):
    """Per-core input dicts."""
    jj = np.arange(128, dtype=np.float32)[:, None]
    ii = np.arange(QSB, dtype=np.float32)[None, :]
    d_pat = np.ascontiguousarray(jj - ii)  # [128, 512]
    patm = np.empty((4, 128, QSB), np.float32)
    patms = np.empty((4, 128, QSB), np.float32)
    for r in range(4):
        v = d_pat + 128.0 * r
        patm[r] = np.where(v <= 0.0, v, MASK_NEG)
        vs = jj + 128.0 * r - 511.0  # slope*(j - i_max), qsb-independent
        patms[r] = np.where(v <= 0.0, vs, MASK_NEG)
    ones_m = np.ones((128, 128), np.float32)

    in_maps = []
    for c in range(N_CORES):
        b, g = divmod(c, HPG)
        xt = np.ascontiguousarray(x[b].T)
        heads = [4 * j + g for j in range(HPG)]  # slot j -> head 4j+g
        hcols = np.concatenate(
            [np.arange(hh * 128, (hh + 1) * 128) for hh in heads]
        )
        qcols = hcols
        kcols = HID + hcols
        vcols = 2 * HID + hcols
        wqk = np.ascontiguousarray(
            np.concatenate([qkv_w[:, qcols], qkv_w[:, kcols]], axis=1)
        )
        wv = np.ascontiguousarray(qkv_w[:, vcols])
        wo = np.ascontiguousarray(out_w[hcols, :])
        bqk_v = np.concatenate([qkv_b[qcols], qkv_b[kcols]])
        bqk = np.ascontiguousarray(bqk_v.reshape(8, 128).T)
        bvb = np.ascontiguousarray(np.broadcast_to(qkv_b[vcols], (128, GF)))
        sl = _SLOPES[heads]
        slopes = np.ascontiguousarray(np.broadcast_to(sl, (128, HPG)))
        mc = np.empty((HPG, 16), np.float32)
        for h in range(HPG):
            for m in range(16):
                mc[h, m] = -128.0 * m * sl[h]
        mconst = np.ascontiguousarray(np.broadcast_to(mc, (128, HPG, 16)))
        jjv = np.arange(128, dtype=np.float32)
        ac = np.empty((128, HPG, 13), np.float32)
        for h in range(HPG):
            for m in range(13):
                ac[:, h, m] = sl[h] * (jjv - 511.0 - 128.0 * m)
        acol_a = np.ascontiguousarray(ac)
        in_maps.append(
            {
                "xt": xt,
                "wqk": wqk,
                "wv": wv,
                "wo": wo,
                "bqk": bqk,
                "bvb": bvb,
                "patd": d_pat,
                "patm": patm,
                "patms": patms,
                "acol": acol_a,
                "slopes": slopes,
                "mconst": mconst,
                "ones_m": ones_m,
            }
        )
    return in_maps


def kernel(x, qkv_w, qkv_b, out_w, out_b):
    x = np.asarray(x, np.float32)
    qkv_w = np.asarray(qkv_w, np.float32)
    qkv_b = np.asarray(qkv_b, np.float32)
    out_w = np.asarray(out_w, np.float32)
    out_b = np.asarray(out_b, np.float32)

    if "nc" not in _NC_CACHE:
        _NC_CACHE["nc"] = _build_nc()
    nc = _NC_CACHE["nc"]

    in_maps = _host_inputs(x, qkv_w, qkv_b, out_w)

    trace = bool(int(os.environ.get("BASS_ATTN_TRACE", "0")))
    if trace:
        import trace_shim  # noqa: F401

    res = run_bass_kernel_spmd(
        nc, in_maps, core_ids=list(range(N_CORES)), trace=trace
    )
    if trace and res.exec_time_ns is not None:
        print(f"HW exec time: {res.exec_time_ns} ns")
        _NC_CACHE["exec_time_ns"] = res.exec_time_ns
        _NC_CACHE["trace"] = (
            res.instructions_and_trace[1] if res.instructions_and_trace else None
        )

    out = np.empty((B, S, HID), np.float32)
    for b in range(B):
        acc = np.zeros((S, HID), np.float64)
        for g in range(HPG):
            acc += res.results[b * HPG + g]["yp"].astype(np.float64)
        out[b] = (acc + out_b.astype(np.float64)).astype(np.float32)
    return out
